# revision 8
# baseline (speedup 1.0000x reference)
"""GATNet forward on 8 TRN2 NeuronCores (Bass/Tile, SPMD) — v3.

Key changes vs baseline:
  - x cast to bf16 on host (halves phase-1 HBM traffic); L-mean via packed
    bf16 add-tree (2x DVE) instead of strided reduce.
  - Node table rows are 512B: int8 xt (scale 2/127, (d,h)-interleaved
    columns) + exact bf16 [s_src|s_dst]. AllGather wire drops 38.5MB ->
    25.7MB and gather traffic ~101MB -> ~64MB per core; the value path
    carries ~0.008 abs quant error, the logit path stays bf16-exact.
  - s_dst aligned per edge via host-built transposed one-hot (St)
    matmuls against an SBUF-resident per-target table (kills the 34MB
    s_dst gather); xt dequantized on the idle ACT engine.
  - Ragged per-block chunk counts (max over cores, shared program).
  - Bias folded in via a ones-row matmul; output stays interleaved on
    device and the host un-interleaves columns at assemble time.
"""
import sys

sys.path.insert(0, "/opt/trn_rl_repo")

import numpy as np
import ml_dtypes

import concourse.bass as bass
import concourse.bacc as bacc
import concourse.mybir as mybir
import concourse.tile as tile
from concourse.masks import make_identity

BF16 = ml_dtypes.bfloat16
FP8 = ml_dtypes.float8_e4m3

f32 = mybir.dt.float32
bf16 = mybir.dt.bfloat16
fp8e4 = mybir.dt.float8e4
i16 = mybir.dt.int16
i8 = mybir.dt.int8
P = 128
SQ = 2.0 / 127.0          # int8 quant scale for xt (|xt|max ~1.83)


def default_cfg():
    return dict(
        N=50000, L=10, CIN=300, HEADS=4, DOUT=64, E=800000, NEG=0.2,
        NCORES=8, G=2,
    )


def derive_cfg(cfg):
    c = dict(cfg)
    c["OUT"] = c["HEADS"] * c["DOUT"]            # 256
    c["XTW"] = c["OUT"] + 2 * c["HEADS"]         # 264: xt | s_src(4) | s_dst(4)
    c["ROWB"] = 512                              # int8 xt + bf16 s pair + pad
    n_per = -(-c["N"] // c["NCORES"])
    c["NP"] = ((n_per + P - 1) // P) * P
    c["NB"] = c["NP"] // P
    c["NPAD"] = c["NP"] * c["NCORES"]
    c["HALF"] = c["NPAD"] // 2
    assert c["HALF"] < 32768
    c["NG"] = -(-c["NB"] // c["G"])
    kt, rem = [], c["CIN"]
    while rem > 0:
        kt.append(min(128, rem))
        rem -= kt[-1]
    c["KT"] = kt
    # (d-major, h-minor) interleave permutation: col d*H+h <- orig h*D+d
    H, D = c["HEADS"], c["DOUT"]
    perm = np.zeros(c["OUT"], np.int64)
    for h in range(H):
        for d in range(D):
            perm[d * H + h] = h * D + d
    c["PERM"] = perm
    return c


def _wrap16(idx, width):
    n = len(idx)
    assert n % 16 == 0
    out = np.zeros((P, width), np.int16)
    w = np.asarray(idx, np.int16).reshape(n // 16, 16).T
    out[:, : n // 16] = np.tile(w, (8, 1))
    return out


def prep_inputs(cfg, x, edge_index, lin_w, lin_b, gat_w, att, gat_bias):
    """Host prep. Returns (in_maps, meta) with ragged per-block chunk info."""
    c = cfg
    N, L, CIN, OUT = c["N"], c["L"], c["CIN"], c["OUT"]
    H, D = c["HEADS"], c["DOUT"]
    NP, NB, HALF, G, NG = c["NP"], c["NB"], c["HALF"], c["G"], c["NG"]
    perm = c["PERM"]

    x = np.asarray(x, np.float32).reshape(N, L * CIN).astype(BF16)
    lin_w = np.asarray(lin_w, np.float32)
    lin_b = np.asarray(lin_b, np.float32)
    gat_w = np.asarray(gat_w, np.float32)
    att = np.asarray(att, np.float32)
    gat_bias = np.asarray(gat_bias, np.float32)

    Ad = np.zeros((OUT, H), np.float32)
    As = np.zeros((OUT, H), np.float32)
    for h in range(H):
        Ad[h * D:(h + 1) * D, h] = att[0, h, 0:D]
        As[h * D:(h + 1) * D, h] = att[0, h, D:2 * D]

    b2 = lin_b @ gat_w                                # [OUT]
    bfull_row = np.concatenate(
        [b2[perm] / SQ, b2 @ As, b2 @ Ad]).astype(BF16)[None, :]
    ones_row = np.ones((1, P), BF16)
    bias_il = np.tile(gat_bias[perm][None, :], (P, 1)).astype(np.float32)

    # edges + self loops, sorted by (target block, lo/hi source half)
    row = np.concatenate([np.asarray(edge_index[0], np.int64), np.arange(N)])
    col = np.concatenate([np.asarray(edge_index[1], np.int64), np.arange(N)])
    core_of = col // NP
    lt = col - core_of * NP
    blk = lt // P
    tin = lt % P
    is_hi = (row >= HALF).astype(np.int64)
    key = (core_of * NB + blk).astype(np.int64)

    order = np.lexsort((is_hi, key))
    key_s = key[order]
    hi_s = is_hi[order]
    tin_s = tin[order]
    ridx_s = (row[order] - hi_s * HALF).astype(np.int16)

    combo = key_s * 2 + hi_s
    change = np.empty(len(combo), bool)
    change[0] = True
    change[1:] = combo[1:] != combo[:-1]
    first = np.nonzero(change)[0]
    runlen = np.diff(np.append(first, len(combo)))
    pos = np.arange(len(combo)) - np.repeat(first, runlen)
    grp = {int(combo[f]): (int(f), int(f + rl)) for f, rl in zip(first, runlen)}

    # ragged chunk counts per (block): max over cores
    CLb = np.zeros(NB, np.int64)
    CHb = np.zeros(NB, np.int64)
    for cid in range(c["NCORES"]):
        for b in range(NB):
            k = cid * NB + b
            lo, hi = grp.get(k * 2, (0, 0))
            CLb[b] = max(CLb[b], -(-(hi - lo) // P))
            lo, hi = grp.get(k * 2 + 1, (0, 0))
            CHb[b] = max(CHb[b], -(-(hi - lo) // P))
    CLb = np.maximum(CLb, 1)
    CHb = np.maximum(CHb, 1)
    CTb = CLb + CHb
    OFF_S = np.concatenate([[0], np.cumsum(CTb)])      # chunk offsets
    OFF_L = np.concatenate([[0], np.cumsum(CLb)])
    OFF_H = np.concatenate([[0], np.cumsum(CHb)])
    SW = int(OFF_S[-1])                                # total chunks per core
    LW = int(OFF_L[-1])
    HW = int(OFF_H[-1])

    meta = dict(CLb=CLb.tolist(), CHb=CHb.tolist(), CTb=CTb.tolist(),
                OFF_S=OFF_S.tolist(), OFF_L=OFF_L.tolist(),
                OFF_H=OFF_H.tolist(), SW=SW, LW=LW, HW=HW)

    in_maps = []
    for cid in range(c["NCORES"]):
        n0 = cid * NP
        xs = np.zeros((NP, L * CIN), BF16)
        hi_n = min(N, n0 + NP)
        if hi_n > n0:
            xs[: hi_n - n0] = x[n0:hi_n]

        S = np.zeros((P, SW * P), FP8)
        St = np.zeros((P, SW * P), FP8)
        ilo = np.zeros((P, LW * 8), np.int16)
        ihi = np.zeros((P, HW * 8), np.int16)
        for b in range(NB):
            k = cid * NB + b
            for half, (offc, offi, arr) in enumerate((
                    (OFF_S[b], OFF_L[b], ilo),
                    (OFF_S[b] + CLb[b], OFF_H[b], ihi))):
                lo, hi = grp.get(k * 2 + half, (0, 0))
                cnt = hi - lo
                nch = int((CLb[b] if half == 0 else CHb[b]))
                t = tin_s[lo:hi]
                pp = pos[lo:hi]
                S[pp % P, (offc + pp // P) * P + t] = 1.0
                St[t, (offc + pp // P) * P + pp % P] = 1.0
                pv = np.zeros(nch * P, np.int16)
                pv[:cnt] = ridx_s[lo:hi]
                arr[:, offi * 8:(offi + nch) * 8] = _wrap16(pv, nch * 8)

        im = {
            "x": xs,
            "lin_wT": np.ascontiguousarray(lin_w.T),
            "gat_w_il": np.ascontiguousarray(gat_w[:, perm]),
            "gat_wT": np.ascontiguousarray(gat_w.T),
            "Ad": Ad, "As": As,
            "bfull_row": bfull_row, "ones_row": ones_row,
            "bias_il": bias_il,
            "S": S, "St": St, "ilo": ilo, "ihi": ihi,
        }
        in_maps.append(im)
    return in_maps, meta


# ---------------------------------------------------------------------------
# device kernel builder
# ---------------------------------------------------------------------------

def build_nc(cfg, meta, no_collective=False, repeat=1, pool_every=0):
    c = cfg
    L, CIN, OUT, XTW = c["L"], c["CIN"], c["OUT"], c["XTW"]
    NP, NB, NPAD, HALF = c["NP"], c["NB"], c["NPAD"], c["HALF"]
    G, NG = c["G"], c["NG"]
    KT = c["KT"]
    NK = len(KT)
    H, D = c["HEADS"], c["DOUT"]
    NEG = c["NEG"]
    co_t = [min(128, OUT - i) for i in range(0, OUT, 128)]
    CLb, CHb, CTb = meta["CLb"], meta["CHb"], meta["CTb"]
    OFF_S, OFF_L, OFF_H = meta["OFF_S"], meta["OFF_L"], meta["OFF_H"]
    SW, LW, HWW = meta["SW"], meta["LW"], meta["HW"]
    CTMAX = max(CTb[g * G:(g + 1) * G][0] + (CTb[g * G + 1] if g * G + 1 < NB else 0)
                for g in range(NG))  # max chunks per group (both blocks)
    CTBMAX = max(CTb)

    nc = bacc.Bacc(num_devices=c["NCORES"])
    x_ext = nc.declare_dram_parameter("x", [NP, L * CIN], bf16, isOutput=False)
    lwT_ext = nc.declare_dram_parameter("lin_wT", [CIN, CIN], f32, isOutput=False)
    gwi_ext = nc.declare_dram_parameter("gat_w_il", [CIN, OUT], f32, isOutput=False)
    gwT_ext = nc.declare_dram_parameter("gat_wT", [OUT, CIN], f32, isOutput=False)
    ad_ext = nc.declare_dram_parameter("Ad", [OUT, H], f32, isOutput=False)
    as_ext = nc.declare_dram_parameter("As", [OUT, H], f32, isOutput=False)
    bfr_ext = nc.declare_dram_parameter("bfull_row", [1, XTW], bf16, isOutput=False)
    ones_ext = nc.declare_dram_parameter("ones_row", [1, P], bf16, isOutput=False)
    bias_ext = nc.declare_dram_parameter("bias_il", [P, OUT], f32, isOutput=False)
    s_ext = nc.declare_dram_parameter("S", [P, SW * P], fp8e4, isOutput=False)
    st_ext = nc.declare_dram_parameter("St", [P, SW * P], fp8e4, isOutput=False)
    ilo_ext = nc.declare_dram_parameter("ilo", [P, LW * 8], i16, isOutput=False)
    ihi_ext = nc.declare_dram_parameter("ihi", [P, HWW * 8], i16, isOutput=False)
    out_ext = nc.declare_dram_parameter("out", [NP, OUT], f32, isOutput=True)

    ROWB = c["ROWB"]
    PKB = OUT + 4 * H                # 272: packed row bytes on the wire
    xts_loc = nc.dram_tensor("xts_loc", [NP, PKB], i8)
    xts_pack = nc.dram_tensor("xts_pack", [NPAD, PKB], i8, addr_space="Shared")
    xts_all = nc.dram_tensor("xts_all", [NPAD, ROWB], i8)

    with tile.TileContext(nc) as tc:
        with tc.tile_pool(name="persist", bufs=1) as pp:
            ident = pp.tile([P, P], bf16)
            make_identity(nc, ident[:])
            ones_t = pp.tile([1, P], bf16)
            nc.sync.dma_start(out=ones_t[:], in_=ones_ext[:])
            bfr_t = pp.tile([1, XTW], bf16)
            nc.sync.dma_start(out=bfr_t[:], in_=bfr_ext[:])
            bias_t = pp.tile([P, OUT], f32)
            nc.sync.dma_start(out=bias_t[:], in_=bias_ext[:])
            sdst_sb = pp.tile([P, NB * H], bf16)
            wfull = [pp.tile([KT[k], XTW], bf16, name=f"wfull{k}")
                     for k in range(NK)]

            # ---------------- preamble: fold weights on device ----------------
            with tc.tile_pool(name="pre", bufs=2) as pre, \
                 tc.tile_pool(name="prep", bufs=1, space="PSUM") as prep:
                lwT = {}
                k0 = 0
                for k in range(NK):
                    m0 = 0
                    for m in range(NK):
                        t = pre.tile([KT[k], KT[m]], f32, name=f"lwT{k}{m}", bufs=1)
                        nc.sync.dma_start(
                            out=t[:], in_=lwT_ext[k0:k0 + KT[k], m0:m0 + KT[m]])
                        lwT[(k, m)] = t
                        m0 += KT[m]
                    k0 += KT[k]
                gw = []
                k0 = 0
                for k in range(NK):
                    t = pre.tile([KT[k], OUT], f32, name=f"gw{k}", bufs=1)
                    nc.sync.dma_start(out=t[:], in_=gwi_ext[k0:k0 + KT[k], :])
                    gw.append(t)
                    k0 += KT[k]
                gwT = {}
                q0 = 0
                for q in range(len(co_t)):
                    m0 = 0
                    for m in range(NK):
                        t = pre.tile([co_t[q], KT[m]], f32, name=f"gwT{q}{m}", bufs=1)
                        nc.sync.dma_start(
                            out=t[:], in_=gwT_ext[q0:q0 + co_t[q], m0:m0 + KT[m]])
                        gwT[(q, m)] = t
                        m0 += KT[m]
                    q0 += co_t[q]
                ad_t, as_t = [], []
                q0 = 0
                for q in range(len(co_t)):
                    t1 = pre.tile([co_t[q], H], f32, name=f"ad{q}", bufs=1)
                    nc.sync.dma_start(out=t1[:], in_=ad_ext[q0:q0 + co_t[q], :])
                    t2 = pre.tile([co_t[q], H], f32, name=f"as{q}", bufs=1)
                    nc.sync.dma_start(out=t2[:], in_=as_ext[q0:q0 + co_t[q], :])
                    ad_t.append(t1)
                    as_t.append(t2)
                    q0 += co_t[q]

                gd_sb, gs_sb = [], []
                for bi, (amat, dst) in enumerate(((ad_t, gd_sb), (as_t, gs_sb))):
                    for m in range(NK):
                        ps = prep.tile([KT[m], H], f32, space="PSUM",
                                       name="gps", uniquify=True)
                        for q in range(len(co_t)):
                            nc.tensor.matmul(
                                ps[:], lhsT=gwT[(q, m)][:], rhs=amat[q][:],
                                start=(q == 0), stop=(q == len(co_t) - 1))
                        sbt = pre.tile([KT[m], H], f32, name=f"gsb{bi}{m}",
                                       bufs=1)
                        nc.scalar.copy(sbt[:], ps[:])
                        dst.append(sbt)

                for m in range(NK):
                    w2ps = prep.tile([KT[m], OUT], f32, space="PSUM",
                                     name="w2ps", uniquify=True)
                    for k in range(NK):
                        nc.tensor.matmul(w2ps[:], lhsT=lwT[(k, m)][:], rhs=gw[k][:],
                                         start=(k == 0), stop=(k == NK - 1))
                    wsps = prep.tile([KT[m], H], f32, space="PSUM",
                                     name="wsps", uniquify=True)
                    for k in range(NK):
                        nc.tensor.matmul(wsps[:], lhsT=lwT[(k, m)][:],
                                         rhs=gs_sb[k][:],
                                         start=(k == 0), stop=(k == NK - 1))
                    wdps = prep.tile([KT[m], H], f32, space="PSUM",
                                     name="wdps", uniquify=True)
                    for k in range(NK):
                        nc.tensor.matmul(wdps[:], lhsT=lwT[(k, m)][:],
                                         rhs=gd_sb[k][:],
                                         start=(k == 0), stop=(k == NK - 1))
                    # xt part lands in quant units (1/(L*SQ)); s parts 1/L
                    nc.scalar.mul(wfull[m][:, 0:OUT], w2ps[:], 1.0 / (L * SQ))
                    nc.scalar.mul(wfull[m][:, OUT:OUT + H], wsps[:], 1.0 / L)
                    nc.scalar.mul(wfull[m][:, OUT + H:XTW], wdps[:], 1.0 / L)

            for rep in range(repeat):
                # ---------------- phase 1 ----------------
                with tc.tile_pool(name="p1", bufs=4) as p1, \
                     tc.tile_pool(name="p1p", bufs=2, space="PSUM") as p1p, \
                     tc.tile_pool(name="p1pt", bufs=3, space="PSUM") as p1pt:
                    for b in range(NB):
                        x_t = p1.tile([P, L * CIN], bf16, name="x_t")
                        nc.sync.dma_start(
                            out=x_t[:], in_=x_ext[b * P:(b + 1) * P, :])
                        # L-mean via packed bf16 add tree (L=10):
                        # s1[0:1200] = x[0:1200]+x[1200:2400] (l0..3 + l4..7)
                        s1 = p1.tile([P, 4 * CIN], bf16, name="s1")
                        nc.vector.tensor_add(
                            s1[:], x_t[:, 0:4 * CIN], x_t[:, 4 * CIN:8 * CIN])
                        s2 = p1.tile([P, 2 * CIN], bf16, name="s2")
                        nc.vector.tensor_add(
                            s2[:], s1[:, 0:2 * CIN], s1[:, 2 * CIN:4 * CIN])
                        xm = p1.tile([P, CIN], bf16, name="xm")
                        nc.vector.tensor_add(
                            xm[:], s2[:, 0:CIN], s2[:, CIN:2 * CIN])
                        nc.vector.tensor_add(
                            xm[:], xm[:], x_t[:, 8 * CIN:9 * CIN])
                        nc.vector.tensor_add(
                            xm[:], xm[:], x_t[:, 9 * CIN:10 * CIN])
                        xt_ps = p1p.tile([P, XTW], f32, space="PSUM", name="xt_ps")
                        k0 = 0
                        for k in range(NK):
                            tr_ps = p1pt.tile([KT[k], P], bf16, space="PSUM",
                                              name="tr_ps")
                            nc.tensor.transpose(
                                tr_ps[:], xm[:, k0:k0 + KT[k]], ident[:])
                            xmT = p1.tile([KT[k], P], bf16, name="xmT")
                            nc.scalar.copy(xmT[:], tr_ps[:])
                            nc.tensor.matmul(
                                xt_ps[:], lhsT=xmT[:], rhs=wfull[k][:],
                                start=(k == 0), stop=False)
                            k0 += KT[k]
                        nc.tensor.matmul(
                            xt_ps[:], lhsT=ones_t[:], rhs=bfr_t[:],
                            start=False, stop=True)
                        xq = p1.tile([P, ROWB], i8, name="xq")
                        nc.scalar.copy(xq[:, 0:OUT], xt_ps[:, 0:OUT])
                        nc.scalar.copy(
                            xq[:, OUT:OUT + 4 * H].bitcast(bf16),
                            xt_ps[:, OUT:XTW])
                        nc.scalar.copy(
                            sdst_sb[:, b * H:(b + 1) * H],
                            xt_ps[:, OUT + H:XTW])
                        nc.scalar.dma_start(
                            out=xts_loc[b * P:(b + 1) * P, :],
                            in_=xq[:, 0:PKB])

                if no_collective:
                    nc.sync.dma_start(out=xts_pack[0:NP, :], in_=xts_loc[:])
                else:
                    nc.gpsimd.collective_compute(
                        "AllGather", mybir.AluOpType.bypass,
                        replica_groups=[list(range(c["NCORES"]))],
                        ins=[xts_loc[:]], outs=[xts_pack[:]])
                # repack 272B wire rows -> 512B-stride gather table
                NQ = 4
                step = NPAD // NQ
                for qq in range(NQ):
                    nc.sync.dma_start(
                        out=xts_all[qq * step:(qq + 1) * step, 0:PKB],
                        in_=xts_pack[qq * step:(qq + 1) * step, :])

                # ---------------- phase 2 ----------------
                with tc.tile_pool(name="p2", bufs=2) as p2, \
                     tc.tile_pool(name="p2p", bufs=3, space="PSUM") as p2p, \
                     tc.tile_pool(name="p2ps", bufs=2, space="PSUM") as p2ps:
                    for g in range(NG):
                        b0 = g * G
                        nb = min(NB - b0, G)
                        bl = list(range(b0, b0 + nb))
                        ncl = sum(CLb[b] for b in bl)
                        nch = sum(CHb[b] for b in bl)
                        nct = ncl + nch
                        il_t = p2.tile([P, G * CTBMAX * 8], i16, name="il_t")
                        nc.sync.dma_start(
                            out=il_t[:, :ncl * 8],
                            in_=ilo_ext[:, OFF_L[b0] * 8:(OFF_L[b0] + ncl) * 8])
                        ih_t = p2.tile([P, G * CTBMAX * 8], i16, name="ih_t")
                        nc.sync.dma_start(
                            out=ih_t[:, :nch * 8],
                            in_=ihi_ext[:, OFF_H[b0] * 8:(OFF_H[b0] + nch) * 8])
                        xg_lo = p2.tile([P, G * CTBMAX * ROWB], i8,
                                        name="xg_lo", bufs=2)
                        nc.gpsimd.dma_gather(
                            out_ap=xg_lo[:, :ncl * ROWB].rearrange(
                                "p (c e) -> p c e", e=ROWB),
                            in_ap=xts_all[0:HALF, :],
                            idxs_ap=il_t[:, :ncl * 8],
                            num_idxs=ncl * P, num_idxs_reg=ncl * P,
                            elem_size=ROWB, single_packet=False)
                        xg_hi = p2.tile([P, G * CTBMAX * ROWB], i8,
                                        name="xg_hi", bufs=2)
                        nc.gpsimd.dma_gather(
                            out_ap=xg_hi[:, :nch * ROWB].rearrange(
                                "p (c e) -> p c e", e=ROWB),
                            in_ap=xts_all[HALF:NPAD, :],
                            idxs_ap=ih_t[:, :nch * 8],
                            num_idxs=nch * P, num_idxs_reg=nch * P,
                            elem_size=ROWB, single_packet=False)
                        s_t = p2.tile([P, G * CTBMAX * P], fp8e4, name="s_t")
                        nc.sync.dma_start(
                            out=s_t[:, :nct * P],
                            in_=s_ext[:, OFF_S[b0] * P:(OFF_S[b0] + nct) * P])
                        st_t = p2.tile([P, G * CTBMAX * P], fp8e4, name="st_t")
                        nc.sync.dma_start(
                            out=st_t[:, :nct * P],
                            in_=st_ext[:, OFF_S[b0] * P:(OFF_S[b0] + nct) * P])

                        lo_off = 0
                        hi_off = 0
                        for j in range(nb):
                            b = b0 + j
                            cl, ch = CLb[b], CHb[b]
                            ct = cl + ch
                            soff = (OFF_S[b] - OFF_S[b0]) * P
                            # dequant int8 -> bf16 (ACT), lo | hi concat
                            xgl3 = xg_lo[:, lo_off * ROWB:(lo_off + cl) * ROWB
                                         ].rearrange("p (c e) -> p c e", e=ROWB)
                            xgh3 = xg_hi[:, hi_off * ROWB:(hi_off + ch) * ROWB
                                         ].rearrange("p (c e) -> p c e", e=ROWB)
                            xb = p2.tile([P, CTBMAX * OUT], bf16, name="xb",
                                         bufs=3)
                            nc.scalar.mul(
                                xb[:, 0:cl * OUT].rearrange(
                                    "p (c e) -> p c e", e=OUT),
                                xgl3[:, :, 0:OUT], SQ)
                            nc.scalar.mul(
                                xb[:, cl * OUT:ct * OUT].rearrange(
                                    "p (c e) -> p c e", e=OUT),
                                xgh3[:, :, 0:OUT], SQ)
                            ve = (nc.gpsimd if pool_every and
                                  b % pool_every == pool_every - 1
                                  else nc.vector)
                            # s_dst per position: St one-hot matmuls
                            sd_ps = p2ps.tile([P, CTBMAX * H], f32, space="PSUM",
                                              name="sd_ps")
                            for cc in range(ct):
                                nc.tensor.matmul(
                                    sd_ps[:, cc * H:(cc + 1) * H],
                                    lhsT=st_t[:, soff + cc * P:soff + (cc + 1) * P],
                                    rhs=sdst_sb[:, b * H:(b + 1) * H],
                                    start=True, stop=True)
                            # logits = gathered s_src (exact bf16) + s_dst
                            lg = p2.tile([P, CTBMAX * H], f32, name="lg")
                            lg3 = lg[:].rearrange("p (c h) -> p c h", h=H)
                            sd3 = sd_ps[:].rearrange("p (c h) -> p c h", h=H)
                            nc.vector.tensor_add(
                                lg3[:, 0:cl, :],
                                xgl3[:, :, OUT:OUT + 2 * H].bitcast(bf16)[:, :, 0:H],
                                sd3[:, 0:cl, :])
                            nc.vector.tensor_add(
                                lg3[:, cl:ct, :],
                                xgh3[:, :, OUT:OUT + 2 * H].bitcast(bf16)[:, :, 0:H],
                                sd3[:, cl:ct, :])
                            # leaky relu + exp
                            lgm = p2.tile([P, CTBMAX * H], f32, name="lgm")
                            nc.vector.tensor_scalar_mul(
                                lgm[:, 0:ct * H], lg[:, 0:ct * H], NEG)
                            nc.vector.tensor_tensor(
                                out=lg[:, 0:ct * H], in0=lgm[:, 0:ct * H],
                                in1=lg[:, 0:ct * H], op=mybir.AluOpType.max)
                            w_bf = p2.tile([P, CTBMAX * H], bf16, name="w_bf")
                            nc.scalar.activation(
                                w_bf[:, 0:ct * H], lg[:, 0:ct * H],
                                mybir.ActivationFunctionType.Exp)
                            # rhs = [w * xt | w] (value part (d,h)-interleaved)
                            rhs = p2.tile([P, CTBMAX * (OUT + H)], bf16,
                                          name="rhs", bufs=3)
                            rhs3 = rhs[:].rearrange("p (c e) -> p c e", e=OUT + H)
                            w3 = w_bf[:].rearrange("p (c h) -> p c h", h=H)
                            ve.tensor_mul(
                                rhs3[:, 0:ct, 0:OUT].rearrange(
                                    "p c (d h) -> p c d h", h=H),
                                xb[:, 0:ct * OUT].rearrange(
                                    "p (c d h) -> p c d h", d=D, h=H),
                                w3[:, 0:ct, None, :].to_broadcast([P, ct, D, H]))
                            nc.vector.tensor_copy(
                                rhs3[:, 0:ct, OUT:OUT + H], w3[:, 0:ct, :])
                            # accumulate [numer | denom]
                            ps_b = p2p.tile([P, OUT + H], f32, space="PSUM",
                                            name="ps_b")
                            for cc in range(ct):
                                nc.tensor.matmul(
                                    ps_b[:],
                                    lhsT=s_t[:, soff + cc * P:soff + (cc + 1) * P],
                                    rhs=rhs3[:, cc, :],
                                    start=(cc == 0), stop=(cc == ct - 1))
                            rd = p2.tile([P, H], f32, name="rd")
                            nc.vector.reciprocal(rd[:], ps_b[:, OUT:OUT + H])
                            outv = p2.tile([P, OUT], f32, name="outv")
                            nc.vector.tensor_mul(
                                outv[:].rearrange("p (d h) -> p d h", h=H),
                                ps_b[:, 0:OUT].rearrange("p (d h) -> p d h", h=H),
                                rd[:, None, :].to_broadcast([P, D, H]))
                            nc.vector.tensor_add(outv[:], outv[:], bias_t[:])
                            nc.scalar.dma_start(
                                out=out_ext[b * P:(b + 1) * P, :], in_=outv[:])
                            lo_off += cl
                            hi_off += ch

    nc.finalize()
    return nc


# ---------------------------------------------------------------------------
# entry points
# ---------------------------------------------------------------------------

def run_spmd(nc, in_maps, cfg, trace=False):
    from concourse.bass_utils import run_bass_kernel_spmd

    return run_bass_kernel_spmd(
        nc, in_maps, list(range(cfg["NCORES"])), trace=trace)


def assemble_output(cfg, results):
    perm = cfg["PERM"]
    inv = np.argsort(perm)
    out = np.zeros((cfg["N"], cfg["OUT"]), np.float32)
    for cid in range(cfg["NCORES"]):
        n0 = cid * cfg["NP"]
        n1 = min(cfg["N"], n0 + cfg["NP"])
        if n1 > n0:
            out[n0:n1] = results[cid]["out"][0:n1 - n0][:, inv]
    return out


def run_full(inputs, trace=False):
    cfg = derive_cfg(default_cfg())
    in_maps, meta = prep_inputs(
        cfg, inputs["x"], inputs["edge_index"], inputs["lin_w"],
        inputs["lin_b"], inputs["gat_w"], inputs["att"], inputs["gat_bias"])
    nc = build_nc(cfg, meta)
    r = run_spmd(nc, in_maps, cfg, trace=trace)
    return assemble_output(cfg, r.results), r


def kernel(**inputs):
    out, _ = run_full(inputs, trace=False)
    return out


# revision 16
# speedup vs baseline: 1.1668x; 1.1668x over previous
"""GATNet forward on 8 TRN2 NeuronCores (Bass/Tile, SPMD) — v3.

Key changes vs baseline:
  - x cast to bf16 on host (halves phase-1 HBM traffic); L-mean via packed
    bf16 add-tree (2x DVE) instead of strided reduce.
  - Node table rows are 512B: int8 xt (scale 2/127, (d,h)-interleaved
    columns) + exact bf16 [s_src|s_dst]. AllGather wire drops 38.5MB ->
    25.7MB and gather traffic ~101MB -> ~64MB per core; the value path
    carries ~0.008 abs quant error, the logit path stays bf16-exact.
  - s_dst aligned per edge via host-built transposed one-hot (St)
    matmuls against an SBUF-resident per-target table (kills the 34MB
    s_dst gather); xt dequantized on the idle ACT engine.
  - Ragged per-block chunk counts (max over cores, shared program).
  - Bias folded in via a ones-row matmul; output stays interleaved on
    device and the host un-interleaves columns at assemble time.
"""
import sys

sys.path.insert(0, "/opt/trn_rl_repo")

import numpy as np
import ml_dtypes

import concourse.bass as bass
import concourse.bacc as bacc
import concourse.mybir as mybir
import concourse.tile as tile
from concourse.masks import make_identity

BF16 = ml_dtypes.bfloat16
FP8 = ml_dtypes.float8_e4m3

f32 = mybir.dt.float32
bf16 = mybir.dt.bfloat16
fp8e4 = mybir.dt.float8e4
i16 = mybir.dt.int16
i8 = mybir.dt.int8
P = 128
SQ = 2.0 / 127.0          # int8 quant scale for xt (|xt|max ~1.83)


def default_cfg():
    return dict(
        N=50000, L=10, CIN=300, HEADS=4, DOUT=64, E=800000, NEG=0.2,
        NCORES=8, G=2,
    )


def derive_cfg(cfg):
    c = dict(cfg)
    c["OUT"] = c["HEADS"] * c["DOUT"]            # 256
    c["XTW"] = c["OUT"] + 2 * c["HEADS"]         # 264: xt | s_src(4) | s_dst(4)
    c["ROWB"] = 512                              # int8 xt + bf16 s pair + pad
    n_per = -(-c["N"] // c["NCORES"])
    c["NP"] = ((n_per + P - 1) // P) * P
    c["NB"] = c["NP"] // P
    c["NPAD"] = c["NP"] * c["NCORES"]
    c["HALF"] = c["NPAD"] // 2
    assert c["HALF"] < 32768
    c["NG"] = -(-c["NB"] // c["G"])
    kt, rem = [], c["CIN"]
    while rem > 0:
        kt.append(min(128, rem))
        rem -= kt[-1]
    c["KT"] = kt
    # (d-major, h-minor) interleave permutation: col d*H+h <- orig h*D+d
    H, D = c["HEADS"], c["DOUT"]
    perm = np.zeros(c["OUT"], np.int64)
    for h in range(H):
        for d in range(D):
            perm[d * H + h] = h * D + d
    c["PERM"] = perm
    return c


def _wrap16(idx, width):
    n = len(idx)
    assert n % 16 == 0
    out = np.zeros((P, width), np.int16)
    w = np.asarray(idx, np.int16).reshape(n // 16, 16).T
    out[:, : n // 16] = np.tile(w, (8, 1))
    return out


def prep_inputs(cfg, x, edge_index, lin_w, lin_b, gat_w, att, gat_bias):
    """Host prep. Returns (in_maps, meta) with ragged per-block chunk info."""
    c = cfg
    N, L, CIN, OUT = c["N"], c["L"], c["CIN"], c["OUT"]
    H, D = c["HEADS"], c["DOUT"]
    NP, NB, HALF, G, NG = c["NP"], c["NB"], c["HALF"], c["G"], c["NG"]
    perm = c["PERM"]

    x = np.asarray(x, np.float32).reshape(N, L * CIN).astype(BF16)
    lin_w = np.asarray(lin_w, np.float32)
    lin_b = np.asarray(lin_b, np.float32)
    gat_w = np.asarray(gat_w, np.float32)
    att = np.asarray(att, np.float32)
    gat_bias = np.asarray(gat_bias, np.float32)

    Ad = np.zeros((OUT, H), np.float32)
    As = np.zeros((OUT, H), np.float32)
    for h in range(H):
        Ad[h * D:(h + 1) * D, h] = att[0, h, 0:D]
        As[h * D:(h + 1) * D, h] = att[0, h, D:2 * D]

    b2 = lin_b @ gat_w                                # [OUT]
    bfull_row = np.concatenate(
        [b2[perm] / SQ, b2 @ As, b2 @ Ad]).astype(BF16)[None, :]
    ones_row = np.ones((1, P), BF16)
    bias_il = np.tile(gat_bias[perm][None, :], (P, 1)).astype(np.float32)

    # edges + self loops, sorted by (target block, lo/hi source half)
    row = np.concatenate([np.asarray(edge_index[0], np.int64), np.arange(N)])
    col = np.concatenate([np.asarray(edge_index[1], np.int64), np.arange(N)])
    core_of = col // NP
    lt = col - core_of * NP
    blk = lt // P
    tin = lt % P
    is_hi = (row >= HALF).astype(np.int64)
    key = (core_of * NB + blk).astype(np.int64)

    order = np.lexsort((is_hi, key))
    key_s = key[order]
    hi_s = is_hi[order]
    tin_s = tin[order]
    ridx_s = (row[order] - hi_s * HALF).astype(np.int16)

    combo = key_s * 2 + hi_s
    change = np.empty(len(combo), bool)
    change[0] = True
    change[1:] = combo[1:] != combo[:-1]
    first = np.nonzero(change)[0]
    runlen = np.diff(np.append(first, len(combo)))
    pos = np.arange(len(combo)) - np.repeat(first, runlen)
    grp = {int(combo[f]): (int(f), int(f + rl)) for f, rl in zip(first, runlen)}

    # ragged chunk counts per (block): max over cores
    CLb = np.zeros(NB, np.int64)
    CHb = np.zeros(NB, np.int64)
    for cid in range(c["NCORES"]):
        for b in range(NB):
            k = cid * NB + b
            lo, hi = grp.get(k * 2, (0, 0))
            CLb[b] = max(CLb[b], -(-(hi - lo) // P))
            lo, hi = grp.get(k * 2 + 1, (0, 0))
            CHb[b] = max(CHb[b], -(-(hi - lo) // P))
    CLb = np.maximum(CLb, 1)
    CHb = np.maximum(CHb, 1)
    CTb = CLb + CHb
    OFF_S = np.concatenate([[0], np.cumsum(CTb)])      # chunk offsets
    OFF_L = np.concatenate([[0], np.cumsum(CLb)])
    OFF_H = np.concatenate([[0], np.cumsum(CHb)])
    SW = int(OFF_S[-1])                                # total chunks per core
    LW = int(OFF_L[-1])
    HW = int(OFF_H[-1])

    meta = dict(CLb=CLb.tolist(), CHb=CHb.tolist(), CTb=CTb.tolist(),
                OFF_S=OFF_S.tolist(), OFF_L=OFF_L.tolist(),
                OFF_H=OFF_H.tolist(), SW=SW, LW=LW, HW=HW)

    in_maps = []
    for cid in range(c["NCORES"]):
        n0 = cid * NP
        xs = np.zeros((NP, L * CIN), BF16)
        hi_n = min(N, n0 + NP)
        if hi_n > n0:
            xs[: hi_n - n0] = x[n0:hi_n]

        S = np.zeros((P, SW * P), FP8)
        St = np.zeros((P, SW * P), FP8)
        ilo = np.zeros((P, LW * 8), np.int16)
        ihi = np.zeros((P, HW * 8), np.int16)
        for b in range(NB):
            k = cid * NB + b
            for half, (offc, offi, arr) in enumerate((
                    (OFF_S[b], OFF_L[b], ilo),
                    (OFF_S[b] + CLb[b], OFF_H[b], ihi))):
                lo, hi = grp.get(k * 2 + half, (0, 0))
                cnt = hi - lo
                nch = int((CLb[b] if half == 0 else CHb[b]))
                t = tin_s[lo:hi]
                pp = pos[lo:hi]
                S[pp % P, (offc + pp // P) * P + t] = 1.0
                St[t, (offc + pp // P) * P + pp % P] = 1.0
                pv = np.zeros(nch * P, np.int16)
                pv[:cnt] = ridx_s[lo:hi]
                arr[:, offi * 8:(offi + nch) * 8] = _wrap16(pv, nch * 8)

        im = {
            "x": xs,
            "lin_wT": np.ascontiguousarray(lin_w.T),
            "gat_w_il": np.ascontiguousarray(gat_w[:, perm]),
            "gat_wT": np.ascontiguousarray(gat_w.T),
            "Ad": Ad, "As": As,
            "bfull_row": bfull_row, "ones_row": ones_row,
            "bias_il": bias_il,
            "S": S, "St": St, "ilo": ilo, "ihi": ihi,
        }
        in_maps.append(im)
    return in_maps, meta


# ---------------------------------------------------------------------------
# device kernel builder
# ---------------------------------------------------------------------------

def build_nc(cfg, meta, no_collective=False, repeat=1, pool_every=0):
    c = cfg
    L, CIN, OUT, XTW = c["L"], c["CIN"], c["OUT"], c["XTW"]
    NP, NB, NPAD, HALF = c["NP"], c["NB"], c["NPAD"], c["HALF"]
    G, NG = c["G"], c["NG"]
    KT = c["KT"]
    NK = len(KT)
    H, D = c["HEADS"], c["DOUT"]
    NEG = c["NEG"]
    co_t = [min(128, OUT - i) for i in range(0, OUT, 128)]
    CLb, CHb, CTb = meta["CLb"], meta["CHb"], meta["CTb"]
    OFF_S, OFF_L, OFF_H = meta["OFF_S"], meta["OFF_L"], meta["OFF_H"]
    SW, LW, HWW = meta["SW"], meta["LW"], meta["HW"]
    CTMAX = max(CTb[g * G:(g + 1) * G][0] + (CTb[g * G + 1] if g * G + 1 < NB else 0)
                for g in range(NG))  # max chunks per group (both blocks)
    CTBMAX = max(CTb)

    nc = bacc.Bacc(num_devices=c["NCORES"])
    x_ext = nc.declare_dram_parameter("x", [NP, L * CIN], bf16, isOutput=False)
    lwT_ext = nc.declare_dram_parameter("lin_wT", [CIN, CIN], f32, isOutput=False)
    gwi_ext = nc.declare_dram_parameter("gat_w_il", [CIN, OUT], f32, isOutput=False)
    gwT_ext = nc.declare_dram_parameter("gat_wT", [OUT, CIN], f32, isOutput=False)
    ad_ext = nc.declare_dram_parameter("Ad", [OUT, H], f32, isOutput=False)
    as_ext = nc.declare_dram_parameter("As", [OUT, H], f32, isOutput=False)
    bfr_ext = nc.declare_dram_parameter("bfull_row", [1, XTW], bf16, isOutput=False)
    ones_ext = nc.declare_dram_parameter("ones_row", [1, P], bf16, isOutput=False)
    bias_ext = nc.declare_dram_parameter("bias_il", [P, OUT], f32, isOutput=False)
    s_ext = nc.declare_dram_parameter("S", [P, SW * P], fp8e4, isOutput=False)
    st_ext = nc.declare_dram_parameter("St", [P, SW * P], fp8e4, isOutput=False)
    ilo_ext = nc.declare_dram_parameter("ilo", [P, LW * 8], i16, isOutput=False)
    ihi_ext = nc.declare_dram_parameter("ihi", [P, HWW * 8], i16, isOutput=False)
    out_ext = nc.declare_dram_parameter("out", [NP, OUT], f32, isOutput=True)

    ROWB = c["ROWB"]
    PKB = OUT + 4 * H                # 272: packed row bytes on the wire
    xts_loc = nc.dram_tensor("xts_loc", [NP, PKB], i8)
    xts_pack = nc.dram_tensor("xts_pack", [NPAD, PKB], i8, addr_space="Shared")
    xts_lo = nc.dram_tensor("xts_lo", [HALF, ROWB], i8)
    xts_hi = nc.dram_tensor("xts_hi", [NPAD - HALF, ROWB], i8)

    with tile.TileContext(nc) as tc:
        with tc.tile_pool(name="persist", bufs=1) as pp:
            ident = pp.tile([P, P], bf16)
            make_identity(nc, ident[:])
            ones_t = pp.tile([1, P], bf16)
            nc.sync.dma_start(out=ones_t[:], in_=ones_ext[:])
            bfr_t = pp.tile([1, XTW], bf16)
            nc.sync.dma_start(out=bfr_t[:], in_=bfr_ext[:])
            bias_t = pp.tile([P, OUT], f32)
            nc.sync.dma_start(out=bias_t[:], in_=bias_ext[:])
            sdst_sb = pp.tile([P, NB * H], bf16)
            wfull = [pp.tile([KT[k], XTW], bf16, name=f"wfull{k}")
                     for k in range(NK)]

            # ---------------- preamble: fold weights on device ----------------
            with tc.tile_pool(name="pre", bufs=2) as pre, \
                 tc.tile_pool(name="prep", bufs=1, space="PSUM") as prep:
                lwT = {}
                k0 = 0
                for k in range(NK):
                    m0 = 0
                    for m in range(NK):
                        t = pre.tile([KT[k], KT[m]], f32, name=f"lwT{k}{m}", bufs=1)
                        nc.sync.dma_start(
                            out=t[:], in_=lwT_ext[k0:k0 + KT[k], m0:m0 + KT[m]])
                        lwT[(k, m)] = t
                        m0 += KT[m]
                    k0 += KT[k]
                gw = []
                k0 = 0
                for k in range(NK):
                    t = pre.tile([KT[k], OUT], f32, name=f"gw{k}", bufs=1)
                    nc.sync.dma_start(out=t[:], in_=gwi_ext[k0:k0 + KT[k], :])
                    gw.append(t)
                    k0 += KT[k]
                gwT = {}
                q0 = 0
                for q in range(len(co_t)):
                    m0 = 0
                    for m in range(NK):
                        t = pre.tile([co_t[q], KT[m]], f32, name=f"gwT{q}{m}", bufs=1)
                        nc.sync.dma_start(
                            out=t[:], in_=gwT_ext[q0:q0 + co_t[q], m0:m0 + KT[m]])
                        gwT[(q, m)] = t
                        m0 += KT[m]
                    q0 += co_t[q]
                ad_t, as_t = [], []
                q0 = 0
                for q in range(len(co_t)):
                    t1 = pre.tile([co_t[q], H], f32, name=f"ad{q}", bufs=1)
                    nc.sync.dma_start(out=t1[:], in_=ad_ext[q0:q0 + co_t[q], :])
                    t2 = pre.tile([co_t[q], H], f32, name=f"as{q}", bufs=1)
                    nc.sync.dma_start(out=t2[:], in_=as_ext[q0:q0 + co_t[q], :])
                    ad_t.append(t1)
                    as_t.append(t2)
                    q0 += co_t[q]

                gd_sb, gs_sb = [], []
                for bi, (amat, dst) in enumerate(((ad_t, gd_sb), (as_t, gs_sb))):
                    for m in range(NK):
                        ps = prep.tile([KT[m], H], f32, space="PSUM",
                                       name="gps", uniquify=True)
                        for q in range(len(co_t)):
                            nc.tensor.matmul(
                                ps[:], lhsT=gwT[(q, m)][:], rhs=amat[q][:],
                                start=(q == 0), stop=(q == len(co_t) - 1))
                        sbt = pre.tile([KT[m], H], f32, name=f"gsb{bi}{m}",
                                       bufs=1)
                        nc.scalar.copy(sbt[:], ps[:])
                        dst.append(sbt)

                for m in range(NK):
                    w2ps = prep.tile([KT[m], OUT], f32, space="PSUM",
                                     name="w2ps", uniquify=True)
                    for k in range(NK):
                        nc.tensor.matmul(w2ps[:], lhsT=lwT[(k, m)][:], rhs=gw[k][:],
                                         start=(k == 0), stop=(k == NK - 1))
                    wsps = prep.tile([KT[m], H], f32, space="PSUM",
                                     name="wsps", uniquify=True)
                    for k in range(NK):
                        nc.tensor.matmul(wsps[:], lhsT=lwT[(k, m)][:],
                                         rhs=gs_sb[k][:],
                                         start=(k == 0), stop=(k == NK - 1))
                    wdps = prep.tile([KT[m], H], f32, space="PSUM",
                                     name="wdps", uniquify=True)
                    for k in range(NK):
                        nc.tensor.matmul(wdps[:], lhsT=lwT[(k, m)][:],
                                         rhs=gd_sb[k][:],
                                         start=(k == 0), stop=(k == NK - 1))
                    # xt part lands in quant units (1/(L*SQ)); s parts 1/L
                    nc.scalar.mul(wfull[m][:, 0:OUT], w2ps[:], 1.0 / (L * SQ))
                    nc.scalar.mul(wfull[m][:, OUT:OUT + H], wsps[:], 1.0 / L)
                    nc.scalar.mul(wfull[m][:, OUT + H:XTW], wdps[:], 1.0 / L)

            for rep in range(repeat):
                # ---------------- phase 1 ----------------
                with tc.tile_pool(name="p1", bufs=4) as p1, \
                     tc.tile_pool(name="p1p", bufs=2, space="PSUM") as p1p, \
                     tc.tile_pool(name="p1pt", bufs=3, space="PSUM") as p1pt:
                    for b in range(NB):
                        x_t = p1.tile([P, L * CIN], bf16, name="x_t")
                        nc.sync.dma_start(
                            out=x_t[:], in_=x_ext[b * P:(b + 1) * P, :])
                        # L-mean via packed bf16 add tree (L=10):
                        # s1[0:1200] = x[0:1200]+x[1200:2400] (l0..3 + l4..7)
                        s1 = p1.tile([P, 4 * CIN], bf16, name="s1")
                        nc.vector.tensor_add(
                            s1[:], x_t[:, 0:4 * CIN], x_t[:, 4 * CIN:8 * CIN])
                        s2 = p1.tile([P, 2 * CIN], bf16, name="s2")
                        nc.vector.tensor_add(
                            s2[:], s1[:, 0:2 * CIN], s1[:, 2 * CIN:4 * CIN])
                        xm = p1.tile([P, CIN], bf16, name="xm")
                        nc.vector.tensor_add(
                            xm[:], s2[:, 0:CIN], s2[:, CIN:2 * CIN])
                        nc.vector.tensor_add(
                            xm[:], xm[:], x_t[:, 8 * CIN:9 * CIN])
                        nc.vector.tensor_add(
                            xm[:], xm[:], x_t[:, 9 * CIN:10 * CIN])
                        xt_ps = p1p.tile([P, XTW], f32, space="PSUM", name="xt_ps")
                        k0 = 0
                        for k in range(NK):
                            tr_ps = p1pt.tile([KT[k], P], bf16, space="PSUM",
                                              name="tr_ps")
                            nc.tensor.transpose(
                                tr_ps[:], xm[:, k0:k0 + KT[k]], ident[:])
                            xmT = p1.tile([KT[k], P], bf16, name="xmT")
                            nc.scalar.copy(xmT[:], tr_ps[:])
                            nc.tensor.matmul(
                                xt_ps[:], lhsT=xmT[:], rhs=wfull[k][:],
                                start=(k == 0), stop=False)
                            k0 += KT[k]
                        nc.tensor.matmul(
                            xt_ps[:], lhsT=ones_t[:], rhs=bfr_t[:],
                            start=False, stop=True)
                        xq = p1.tile([P, ROWB], i8, name="xq")
                        nc.scalar.copy(xq[:, 0:OUT], xt_ps[:, 0:OUT])
                        nc.scalar.copy(
                            xq[:, OUT:OUT + 4 * H].bitcast(bf16),
                            xt_ps[:, OUT:XTW])
                        nc.scalar.copy(
                            sdst_sb[:, b * H:(b + 1) * H],
                            xt_ps[:, OUT + H:XTW])
                        nc.scalar.dma_start(
                            out=xts_loc[b * P:(b + 1) * P, :],
                            in_=xq[:, 0:PKB])

                if no_collective:
                    nc.sync.dma_start(out=xts_pack[0:NP, :], in_=xts_loc[:])
                else:
                    nc.gpsimd.collective_compute(
                        "AllGather", mybir.AluOpType.bypass,
                        replica_groups=[list(range(c["NCORES"]))],
                        ins=[xts_loc[:]], outs=[xts_pack[:]])
                # repack 272B wire rows -> 512B-stride gather tables; lo
                # first so its gathers overlap the hi-half repack
                hstep = HALF // 2
                for qq in range(2):
                    nc.sync.dma_start(
                        out=xts_lo[qq * hstep:(qq + 1) * hstep, 0:PKB],
                        in_=xts_pack[qq * hstep:(qq + 1) * hstep, :])
                for qq in range(2):
                    nc.sync.dma_start(
                        out=xts_hi[qq * hstep:(qq + 1) * hstep, 0:PKB],
                        in_=xts_pack[HALF + qq * hstep:
                                     HALF + (qq + 1) * hstep, :])

                # ---------------- phase 2 ----------------
                with tc.tile_pool(name="p2", bufs=2) as p2, \
                     tc.tile_pool(name="p2p", bufs=3, space="PSUM") as p2p, \
                     tc.tile_pool(name="p2ps", bufs=2, space="PSUM") as p2ps:
                    for g in range(NG):
                        b0 = g * G
                        nb = min(NB - b0, G)
                        bl = list(range(b0, b0 + nb))
                        ncl = sum(CLb[b] for b in bl)
                        nch = sum(CHb[b] for b in bl)
                        nct = ncl + nch
                        il_t = p2.tile([P, G * CTBMAX * 8], i16, name="il_t")
                        nc.sync.dma_start(
                            out=il_t[:, :ncl * 8],
                            in_=ilo_ext[:, OFF_L[b0] * 8:(OFF_L[b0] + ncl) * 8])
                        ih_t = p2.tile([P, G * CTBMAX * 8], i16, name="ih_t")
                        nc.sync.dma_start(
                            out=ih_t[:, :nch * 8],
                            in_=ihi_ext[:, OFF_H[b0] * 8:(OFF_H[b0] + nch) * 8])
                        xg_lo = p2.tile([P, G * CTBMAX * ROWB], i8,
                                        name="xg_lo", bufs=2)
                        nc.gpsimd.dma_gather(
                            out_ap=xg_lo[:, :ncl * ROWB].rearrange(
                                "p (c e) -> p c e", e=ROWB),
                            in_ap=xts_lo[:, :],
                            idxs_ap=il_t[:, :ncl * 8],
                            num_idxs=ncl * P, num_idxs_reg=ncl * P,
                            elem_size=ROWB, single_packet=False)
                        xg_hi = p2.tile([P, G * CTBMAX * ROWB], i8,
                                        name="xg_hi", bufs=2)
                        nc.gpsimd.dma_gather(
                            out_ap=xg_hi[:, :nch * ROWB].rearrange(
                                "p (c e) -> p c e", e=ROWB),
                            in_ap=xts_hi[:, :],
                            idxs_ap=ih_t[:, :nch * 8],
                            num_idxs=nch * P, num_idxs_reg=nch * P,
                            elem_size=ROWB, single_packet=False)
                        s_t = p2.tile([P, G * CTBMAX * P], fp8e4, name="s_t")
                        nc.sync.dma_start(
                            out=s_t[:, :nct * P],
                            in_=s_ext[:, OFF_S[b0] * P:(OFF_S[b0] + nct) * P])
                        st_t = p2.tile([P, G * CTBMAX * P], fp8e4, name="st_t")
                        nc.sync.dma_start(
                            out=st_t[:, :nct * P],
                            in_=st_ext[:, OFF_S[b0] * P:(OFF_S[b0] + nct) * P])

                        lo_off = 0
                        hi_off = 0
                        for j in range(nb):
                            b = b0 + j
                            cl, ch = CLb[b], CHb[b]
                            ct = cl + ch
                            soff = (OFF_S[b] - OFF_S[b0]) * P
                            # dequant int8 -> bf16 (ACT), lo | hi concat
                            xgl3 = xg_lo[:, lo_off * ROWB:(lo_off + cl) * ROWB
                                         ].rearrange("p (c e) -> p c e", e=ROWB)
                            xgh3 = xg_hi[:, hi_off * ROWB:(hi_off + ch) * ROWB
                                         ].rearrange("p (c e) -> p c e", e=ROWB)
                            xb = p2.tile([P, CTBMAX * OUT], bf16, name="xb",
                                         bufs=3)
                            nc.scalar.mul(
                                xb[:, 0:cl * OUT].rearrange(
                                    "p (c e) -> p c e", e=OUT),
                                xgl3[:, :, 0:OUT], SQ)
                            nc.scalar.mul(
                                xb[:, cl * OUT:ct * OUT].rearrange(
                                    "p (c e) -> p c e", e=OUT),
                                xgh3[:, :, 0:OUT], SQ)
                            ve = (nc.gpsimd if pool_every and
                                  b % pool_every == pool_every - 1
                                  else nc.vector)
                            # s_dst per position: St one-hot matmuls
                            sd_ps = p2ps.tile([P, CTBMAX * H], f32, space="PSUM",
                                              name="sd_ps")
                            for cc in range(ct):
                                nc.tensor.matmul(
                                    sd_ps[:, cc * H:(cc + 1) * H],
                                    lhsT=st_t[:, soff + cc * P:soff + (cc + 1) * P],
                                    rhs=sdst_sb[:, b * H:(b + 1) * H],
                                    start=True, stop=True)
                            # logits = gathered s_src (exact bf16) + s_dst
                            lg = p2.tile([P, CTBMAX * H], f32, name="lg")
                            lg3 = lg[:].rearrange("p (c h) -> p c h", h=H)
                            sd3 = sd_ps[:].rearrange("p (c h) -> p c h", h=H)
                            nc.vector.tensor_add(
                                lg3[:, 0:cl, :],
                                xgl3[:, :, OUT:OUT + 2 * H].bitcast(bf16)[:, :, 0:H],
                                sd3[:, 0:cl, :])
                            nc.vector.tensor_add(
                                lg3[:, cl:ct, :],
                                xgh3[:, :, OUT:OUT + 2 * H].bitcast(bf16)[:, :, 0:H],
                                sd3[:, cl:ct, :])
                            # leaky relu + exp
                            lgm = p2.tile([P, CTBMAX * H], f32, name="lgm")
                            nc.vector.tensor_scalar_mul(
                                lgm[:, 0:ct * H], lg[:, 0:ct * H], NEG)
                            nc.vector.tensor_tensor(
                                out=lg[:, 0:ct * H], in0=lgm[:, 0:ct * H],
                                in1=lg[:, 0:ct * H], op=mybir.AluOpType.max)
                            w_bf = p2.tile([P, CTBMAX * H], bf16, name="w_bf")
                            nc.scalar.activation(
                                w_bf[:, 0:ct * H], lg[:, 0:ct * H],
                                mybir.ActivationFunctionType.Exp)
                            # rhs = [w * xt | w] (value part (d,h)-interleaved)
                            rhs = p2.tile([P, CTBMAX * (OUT + H)], bf16,
                                          name="rhs", bufs=3)
                            rhs3 = rhs[:].rearrange("p (c e) -> p c e", e=OUT + H)
                            w3 = w_bf[:].rearrange("p (c h) -> p c h", h=H)
                            ve.tensor_mul(
                                rhs3[:, 0:ct, 0:OUT].rearrange(
                                    "p c (d h) -> p c d h", h=H),
                                xb[:, 0:ct * OUT].rearrange(
                                    "p (c d h) -> p c d h", d=D, h=H),
                                w3[:, 0:ct, None, :].to_broadcast([P, ct, D, H]))
                            nc.vector.tensor_copy(
                                rhs3[:, 0:ct, OUT:OUT + H], w3[:, 0:ct, :])
                            # accumulate [numer | denom]
                            ps_b = p2p.tile([P, OUT + H], f32, space="PSUM",
                                            name="ps_b")
                            for cc in range(ct):
                                nc.tensor.matmul(
                                    ps_b[:],
                                    lhsT=s_t[:, soff + cc * P:soff + (cc + 1) * P],
                                    rhs=rhs3[:, cc, :],
                                    start=(cc == 0), stop=(cc == ct - 1))
                            rd = p2.tile([P, H], f32, name="rd")
                            nc.vector.reciprocal(rd[:], ps_b[:, OUT:OUT + H])
                            outv = p2.tile([P, OUT], f32, name="outv")
                            nc.vector.tensor_mul(
                                outv[:].rearrange("p (d h) -> p d h", h=H),
                                ps_b[:, 0:OUT].rearrange("p (d h) -> p d h", h=H),
                                rd[:, None, :].to_broadcast([P, D, H]))
                            nc.vector.tensor_add(outv[:], outv[:], bias_t[:])
                            nc.scalar.dma_start(
                                out=out_ext[b * P:(b + 1) * P, :], in_=outv[:])
                            lo_off += cl
                            hi_off += ch

    nc.finalize()
    return nc


# ---------------------------------------------------------------------------
# entry points
# ---------------------------------------------------------------------------

def run_spmd(nc, in_maps, cfg, trace=False):
    from concourse.bass_utils import run_bass_kernel_spmd

    return run_bass_kernel_spmd(
        nc, in_maps, list(range(cfg["NCORES"])), trace=trace)


def assemble_output(cfg, results):
    perm = cfg["PERM"]
    inv = np.argsort(perm)
    out = np.zeros((cfg["N"], cfg["OUT"]), np.float32)
    for cid in range(cfg["NCORES"]):
        n0 = cid * cfg["NP"]
        n1 = min(cfg["N"], n0 + cfg["NP"])
        if n1 > n0:
            out[n0:n1] = results[cid]["out"][0:n1 - n0][:, inv]
    return out


def run_full(inputs, trace=False):
    cfg = derive_cfg(default_cfg())
    in_maps, meta = prep_inputs(
        cfg, inputs["x"], inputs["edge_index"], inputs["lin_w"],
        inputs["lin_b"], inputs["gat_w"], inputs["att"], inputs["gat_bias"])
    nc = build_nc(cfg, meta)
    r = run_spmd(nc, in_maps, cfg, trace=trace)
    return assemble_output(cfg, r.results), r


def kernel(**inputs):
    out, _ = run_full(inputs, trace=False)
    return out
